# revision 1
# baseline (speedup 1.0000x reference)
"""AttentionBlock (GroupNorm + single-head self-attention + proj + residual)
Trainium2 Bass/Tile kernel, data-parallel over batch across 8 NeuronCores.

Reference computation (per batch element b of 16; C=512, H=W=32, N=1024):
  h   = GroupNorm(x, 8 groups, eps=1e-5) * gn_scale + gn_bias
  qkv = qkv_w @ h + qkv_b            (1x1 conv == matmul over channels)
  q,k,v = split(qkv); attn = softmax(q^T k / sqrt(C)); o = v @ attn^T
  y   = proj_w @ o + proj_b + x

Layout strategy per core (2 batch elements per core):
  - Everything channel-major [C(part-tiles), N(free)] so all matmuls contract
    over the 128-partition dim with no transposes:
      Q,K  : [c, n]  via lhsT = qkv_w^T column tiles
      V^T  : [n, c]  via lhsT = h n-subtiles, rhs = Wv^T
      S^T  : [m(keys), n(queries)] via lhsT = K m-subtiles, rhs = Q
      E    = exp(S^T / sqrt(C)) (no max-subtract needed: logits ~ N(0,1))
      denom: ones^T @ E (PE column-sum), reciprocal on DVE, broadcast back to
             128 partitions with a K=1 matmul
      O    : [c, n] via lhsT = V^T c-subtiles, rhs = E; scaled by recip on evict
      y    : [c, n] via lhsT = proj_w^T, rhs = O; + residual x on evict
  - K-bias is dropped: it shifts every logit of a query equally, which softmax
    cancels exactly. V-bias is folded into the proj bias on the host.
  - Matmul inputs bf16 (PE full rate), all accumulation fp32 in PSUM,
    GroupNorm stats + softmax denominators fp32.
"""

import sys

for _p in ("/opt/trn_rl_repo",):
    if _p not in sys.path:
        sys.path.insert(0, _p)

import math

import ml_dtypes
import numpy as np

import concourse.bass as bass
import concourse.tile as tile
from concourse import mybir
from concourse.vector_clock import ScopedClock, VectorClock

B, C, H, W = 16, 512, 32, 32
N = H * W  # 1024
NUM_GROUPS = 8
EPS = 1e-5
N_CORES = 8
NB = B // N_CORES  # batches per core = 2
CT = C // 128  # channel partition tiles = 4
NT = N // 128  # pixel partition tiles = 8
NH = N // 512  # free-dim halves = 2
GSIZE = C // NUM_GROUPS  # 64 channels per group
SCALE = 1.0 / math.sqrt(C)

F32 = mybir.dt.float32
BF16 = mybir.dt.bfloat16
BF16_NP = ml_dtypes.bfloat16


# --- workaround: this container's walrus accepts only ONE sync wait on the
# SP CTRL drain that TileContext emits at kernel tail; split it into
# single-wait drains.
def _chunked_drain_and_barrier(self, tick_clock, wait_clock):
    gc = tick_clock.global_clock
    ticks = None
    for _k, v in ScopedClock({None: gc}).items():
        ticks = eval(repr(v).replace("VectorClock", ""))
    assert ticks is not None
    n = len(ticks)
    for i in range(n):
        if ticks[i] <= 0:
            continue
        cticks = [ticks[j] if j == i else 0 for j in range(n)]
        drain_inst = self.nc.sync.drain()
        wait_clock.add_sem_waits(
            drain_inst.ins, ScopedClock({None: VectorClock(cticks)})
        )
    self.nc.all_engine_barrier()
    popped = self.nc._tile_sem_poison_stack.pop()
    assert popped is self._sem_poison
    self.nc.clear_and_free_semaphores(list(self.sems.allocated().values()))
    self.nc.all_engine_barrier()


tile.TileContext._drain_and_barrier = _chunked_drain_and_barrier


def _split_multi_waits(nc: bass.Bass, max_waits: int = 1) -> None:
    """Walrus in this container rejects instructions carrying more than one
    sync wait. Hoist excess waits onto same-engine NoOp carriers placed
    immediately before the instruction (same engine queue -> same blocking
    semantics)."""
    n_split = 0
    for f in nc.m.functions:
        for bb in f.blocks:
            insts = bb.instructions
            new = []
            for inst in insts:
                si = inst.sync_info
                if si is not None and len(si.on_wait) > max_waits:
                    waits = list(si.on_wait)
                    keep = waits[-max_waits:]
                    for w in waits[: -max_waits]:
                        nop = mybir.InstNoOp(
                            name=f"{inst.name}-wsplit{n_split}",
                            engine=inst.engine,
                            bass_nofuse=True,
                            sync_info=mybir.SyncInfo(on_wait=[w], on_update=[]),
                        )
                        new.append(nop)
                        n_split += 1
                    inst.sync_info = mybir.SyncInfo(
                        on_wait=keep, on_update=list(si.on_update)
                    )
                new.append(inst)
            insts[:] = new


def build_nc(q_bias_nonzero: bool, p_bias_nonzero: bool) -> bass.Bass:
    nc = bass.Bass(trn_type="TRN2")

    x_d = nc.dram_tensor("x", [NB, C, N], F32, kind="ExternalInput")
    # bf16 copy of x for the GN/stats path: halves the bytes on the critical
    # startup loads; fp32 x only feeds the final residual add (loaded later)
    xb_d = nc.dram_tensor("xb", [NB, C, N], BF16, kind="ExternalInput")
    wqkvT_d = nc.dram_tensor("wqkvT", [C, 3 * C], BF16, kind="ExternalInput")
    pwT_d = nc.dram_tensor("pwT", [C, C], BF16, kind="ExternalInput")
    # packed per-c-tile vectors: [gnsc, gnbi, qb, pb2]
    vecs_d = nc.dram_tensor("vecs", [CT, 128, 4], F32, kind="ExternalInput")
    # group-average block matrix: bmat[p, p'] = 1/64 if p//64 == p'//64.
    # lhsT for a single matmul that both group-reduces AND broadcasts the
    # GN stats across the partitions of each group (symmetric).
    bmat_d = nc.dram_tensor("bmat", [128, 128], BF16, kind="ExternalInput")
    y_d = nc.dram_tensor("y", [NB, C, N], F32, kind="ExternalOutput")

    xap = x_d.ap()
    xbap = xb_d.ap()
    yap = y_d.ap()

    with tile.TileContext(nc) as tc:
        with (
            tc.tile_pool(name="singles", bufs=1) as singles,
            tc.tile_pool(name="xin", bufs=1) as xin,
            tc.tile_pool(name="stats", bufs=2) as stats,
            tc.tile_pool(name="hp", bufs=2) as hp,
            tc.tile_pool(name="qk", bufs=2) as qkp,
            tc.tile_pool(name="vt", bufs=2) as vtp,
            tc.tile_pool(name="ep", bufs=2) as ep,
            tc.tile_pool(name="op", bufs=2) as opl,
            tc.tile_pool(name="yp", bufs=4) as ypl,
            tc.tile_pool(name="ps_mm", bufs=3, space="PSUM") as ps_mm,
            tc.tile_pool(name="ps_o", bufs=3, space="PSUM") as ps_o,
            tc.tile_pool(name="ps_aux", bufs=2, space="PSUM") as ps_aux,
        ):
            # ---- x loads first: they gate GN stats -> everything. Split
            # across the sync HWDGE queue and the gpsimd SWDGE queues so the
            # 4MB arrives through two paths in parallel.
            xt_all = [[None] * CT for _ in range(NB)]
            xb_all = [[None] * CT for _ in range(NB)]
            # batch 0's bf16 x gets both HWDGE rings to itself (it gates the
            # whole pipeline); its fp32 x (residual only) follows on the same
            # rings; batch 1's x goes on the gpsimd SWDGE path behind the
            # weights (not needed until batch 0's attention tail).
            # tiny GN constants first on gpsimd (they gate the GN chain)
            vecs_sb = []
            for ct in range(CT):
                v = singles.tile([128, 4], F32, tag=f"vecs{ct}")
                nc.gpsimd.dma_start(out=v, in_=vecs_d.ap()[ct])
                vecs_sb.append(v)
            gnsc_sb = [v[:, 0:1] for v in vecs_sb]
            gnbi_sb = [v[:, 1:2] for v in vecs_sb]
            qb_sb = [v[:, 2:3] for v in vecs_sb]
            pb2_sb = [v[:, 3:4] for v in vecs_sb]
            bmat = singles.tile([128, 128], BF16, tag="bmat")
            nc.gpsimd.dma_start(out=bmat, in_=bmat_d.ap())

            x_engs = [nc.sync, nc.scalar, nc.gpsimd, nc.scalar]
            for ct in range(CT):
                t = xin.tile([128, N], BF16, tag=f"xb0_{ct}")
                x_engs[ct].dma_start(out=t, in_=xbap[0, ct * 128 : (ct + 1) * 128, :])
                xb_all[0][ct] = t
            for ct in range(CT):
                t = xin.tile([128, N], F32, tag=f"x0_{ct}")
                eng = nc.sync if ct % 2 == 0 else nc.scalar
                eng.dma_start(out=t, in_=xap[0, ct * 128 : (ct + 1) * 128, :])
                xt_all[0][ct] = t
            wt_sb = []
            pw_sb = []
            for ct in range(CT):
                w = singles.tile([128, 3 * C], BF16, tag=f"wqkv{ct}")
                nc.gpsimd.dma_start(out=w, in_=wqkvT_d.ap()[ct * 128 : (ct + 1) * 128, :])
                wt_sb.append(w)
                p = singles.tile([128, C], BF16, tag=f"pw{ct}")
                nc.gpsimd.dma_start(out=p, in_=pwT_d.ap()[ct * 128 : (ct + 1) * 128, :])
                pw_sb.append(p)
            for ct in range(CT):
                t = xin.tile([128, N], BF16, tag=f"xb1_{ct}")
                nc.gpsimd.dma_start(out=t, in_=xbap[1, ct * 128 : (ct + 1) * 128, :])
                xb_all[1][ct] = t
            for ct in range(CT):
                t = xin.tile([128, N], F32, tag=f"x1_{ct}")
                nc.gpsimd.dma_start(out=t, in_=xap[1, ct * 128 : (ct + 1) * 128, :])
                xt_all[1][ct] = t
            # all-ones lhsT: accumulating ones128.T @ E over m-tiles yields the
            # softmax denominator replicated on every partition (no [1,512]
            # single-partition ops, which run ~4us on DVE).
            ones128 = singles.tile([128, 128], BF16, tag="ones128")
            nc.vector.memset(ones128, 1.0)
            epsb = singles.tile([128, 1], F32, tag="epsb")
            nc.vector.memset(epsb, 1.0 + EPS)

            # ---- PE warm-up: HAM unthrottles after ~3.4us of sustained
            # activity, and real matmuls can only start once GN stats are in
            # (~12us). Burn the wait on dummy matmuls so real work runs at
            # 2.4GHz immediately. N=256 keeps each one cheap if they end up
            # delaying real work (engine queues are in-order).
            warm_rhs = singles.tile([128, 128], BF16, tag="warm_rhs")
            nc.vector.memset(warm_rhs, 0.0)
            warm_ps = ps_aux.tile([1, 128], F32, tag="aux")
            for _wi in range(110):
                nc.tensor.matmul(
                    warm_ps, lhsT=ones128[:, 0:1], rhs=warm_rhs,
                    start=True, stop=True,
                )

            # ---- GroupNorm for both batches up front (h feeds everything;
            # batch 1's h being ready early lets its QKV fill PE gaps in
            # batch 0's attention tail).
            ht_all = [[None] * CT for _ in range(NB)]
            b0_last_apply = None
            for b in range(NB):
                for ct in range(CT):
                    t = xb_all[b][ct]
                    st = stats.tile([128, 2, 6], F32, tag=f"st{ct}")
                    for sub in range(2):
                        bi = nc.vector.bn_stats(
                            out=st[:, sub, :], in_=t[:, sub * 512 : (sub + 1) * 512]
                        )
                        if b == 1 and b0_last_apply is not None:
                            # order-only edge: keep batch 1's stats behind
                            # batch 0's GN on the in-order DVE queue (the
                            # scheduler's DMA model otherwise hoists them,
                            # starving batch 0's h)
                            bass._add_dep_helper(
                                bi.ins, b0_last_apply.ins,
                                reason="b1 stats after b0 GN apply",
                            )
                    mv = stats.tile([128, 2], F32, tag=f"mv{ct}")
                    nc.vector.bn_aggr(out=mv, in_=st)
                    # bf16 stats for the group-average matmul. var is carried
                    # as (var-1) so bf16 rounding acts on a ~0.05-magnitude
                    # value instead of ~1.0 (the +1 is restored in the sqrt
                    # bias below).
                    mqb = stats.tile([128, 3], BF16, tag=f"mqb{ct}")
                    nc.vector.tensor_copy(out=mqb[:, 0:1], in_=mv[:, 0:1])
                    nc.vector.tensor_scalar_add(mqb[:, 1:2], mv[:, 1:2], -1.0)
                    nc.vector.tensor_mul(mqb[:, 2:3], mv[:, 0:1], mv[:, 0:1])
                    # one matmul = group-reduce + broadcast: [mean_g, E(var-1), E(m^2)]
                    gps = ps_aux.tile([128, 3], F32, tag="aux")
                    nc.tensor.matmul(gps, lhsT=bmat, rhs=mqb, start=True, stop=True)
                    gs = stats.tile([128, 3], F32, tag=f"gs{ct}")
                    nc.vector.tensor_copy(out=gs, in_=gps)
                    var = stats.tile([128, 1], F32, tag=f"var{ct}")
                    m2 = stats.tile([128, 1], F32, tag=f"m2{ct}")
                    nc.vector.tensor_add(var, gs[:, 1:2], gs[:, 2:3])
                    nc.vector.tensor_mul(m2, gs[:, 0:1], gs[:, 0:1])
                    nc.vector.tensor_sub(var, var, m2)
                    # std = sqrt((var-1 partial) + (1+eps))
                    nc.scalar.activation(
                        out=var, in_=var, func=mybir.ActivationFunctionType.Sqrt,
                        bias=epsb, scale=1.0,
                    )
                    nc.vector.reciprocal(out=var, in_=var)  # rstd
                    A = stats.tile([128, 1], F32, tag=f"A{ct}")
                    Bt = stats.tile([128, 1], F32, tag=f"B{ct}")
                    nc.vector.tensor_mul(A, var, gnsc_sb[ct])
                    nc.vector.tensor_mul(Bt, gs[:, 0:1], A)
                    nc.vector.tensor_sub(Bt, gnbi_sb[ct], Bt)
                    h = hp.tile([128, N], BF16, tag=f"h{ct}")
                    ap_i = nc.vector.tensor_scalar(
                        out=h, in0=t, scalar1=A, scalar2=Bt,
                        op0=mybir.AluOpType.mult, op1=mybir.AluOpType.add,
                    )
                    if b == 0:
                        b0_last_apply = ap_i
                    ht_all[b][ct] = h

            for b in range(NB):
                xt = xt_all[b]
                ht = ht_all[b]

                # ---------- Q, K (channel-major) ----------
                q_sb = []
                k_sb = []
                for qk, off, lst in (("q", 0, q_sb), ("k", C, k_sb)):
                    for co in range(CT):
                        dst = qkp.tile([128, N], BF16, tag=f"{qk}{co}")
                        for nh in range(NH):
                            ps = ps_mm.tile([128, 512], F32, tag="mm")
                            for ct in range(CT):
                                nc.tensor.matmul(
                                    ps,
                                    lhsT=wt_sb[ct][:, off + co * 128 : off + (co + 1) * 128],
                                    rhs=ht[ct][:, nh * 512 : (nh + 1) * 512],
                                    start=(ct == 0),
                                    stop=(ct == CT - 1),
                                )
                            dslice = dst[:, nh * 512 : (nh + 1) * 512]
                            if qk == "q" and q_bias_nonzero:
                                nc.scalar.activation(
                                    out=dslice, in_=ps,
                                    func=mybir.ActivationFunctionType.Identity,
                                    bias=qb_sb[co],
                                )
                            else:
                                nc.scalar.copy(out=dslice, in_=ps)
                        lst.append(dst)

                # ---------- V^T : [n, c] ----------
                vt_sb = []
                for nt in range(NT):
                    ps = ps_mm.tile([128, 512], F32, tag="mm")
                    for ct in range(CT):
                        nc.tensor.matmul(
                            ps,
                            lhsT=ht[ct][:, nt * 128 : (nt + 1) * 128],
                            rhs=wt_sb[ct][:, 2 * C : 3 * C],
                            start=(ct == 0),
                            stop=(ct == CT - 1),
                        )
                    vt = vtp.tile([128, 512], BF16, tag=f"vt{nt}")
                    nc.scalar.copy(out=vt, in_=ps)
                    vt_sb.append(vt)

                # ---------- attention ----------
                # Emission order pipelines the two n-halves: S_T/exp/denom and
                # O-accumulation of half 1 are emitted before half 0's
                # normalize+proj so the PE always has independent matmuls
                # in-stream while a denominator chain resolves.
                es_h = [None] * NH
                dps_h = [None] * NH
                ops_h = [None] * NH

                def st_exp_denom(nh):
                    es = []
                    dps = ps_aux.tile([128, 512], F32, tag="aux")
                    for mt in range(NT):
                        sps = ps_mm.tile([128, 512], F32, tag="mm")
                        for ck in range(CT):
                            nc.tensor.matmul(
                                sps,
                                lhsT=k_sb[ck][:, mt * 128 : (mt + 1) * 128],
                                rhs=q_sb[ck][:, nh * 512 : (nh + 1) * 512],
                                start=(ck == 0),
                                stop=(ck == CT - 1),
                            )
                        e = ep.tile([128, 512], BF16, tag=f"e{nh}_{mt}")
                        nc.scalar.activation(
                            out=e, in_=sps,
                            func=mybir.ActivationFunctionType.Exp, scale=SCALE,
                        )
                        es.append(e)
                        # denominator, replicated across partitions by the
                        # all-ones stationary operand
                        nc.tensor.matmul(
                            dps, lhsT=ones128, rhs=e,
                            start=(mt == 0), stop=(mt == NT - 1),
                        )
                    es_h[nh] = es
                    dps_h[nh] = dps

                def o_accum(nh):
                    es = es_h[nh]
                    o_ps = []
                    for ct4 in range(CT):
                        ops_ = ps_o.tile([128, 512], F32, tag="o")
                        for mt in range(NT):
                            nc.tensor.matmul(
                                ops_,
                                lhsT=vt_sb[mt][:, ct4 * 128 : (ct4 + 1) * 128],
                                rhs=es[mt],
                                start=(mt == 0),
                                stop=(mt == NT - 1),
                            )
                        o_ps.append(ops_)
                    ops_h[nh] = o_ps

                def norm_proj(nh):
                    # note: reciprocal_approx_fast (custom DVE) fails this
                    # walrus's codegen ("ISA wrong length"); the exact one
                    # costs ~3.4us but is hidden under the other n-half's
                    # O-accumulation matmuls by the emission order below
                    rdb = stats.tile([128, 512], F32, tag="rdb")
                    nc.vector.reciprocal(out=rdb, in_=dps_h[nh])
                    o_sb = []
                    for ct4 in range(CT):
                        o = opl.tile([128, 512], BF16, tag=f"o{ct4}")
                        nc.vector.tensor_mul(o, ops_h[nh][ct4], rdb)
                        o_sb.append(o)
                    for cot in range(CT):
                        yps = ps_mm.tile([128, 512], F32, tag="mm")
                        for ct4 in range(CT):
                            nc.tensor.matmul(
                                yps,
                                lhsT=pw_sb[ct4][:, cot * 128 : (cot + 1) * 128],
                                rhs=o_sb[ct4],
                                start=(ct4 == 0),
                                stop=(ct4 == CT - 1),
                            )
                        yo = ypl.tile([128, 512], F32, tag="y")
                        if p_bias_nonzero:
                            nc.scalar.activation(
                                out=yo, in_=yps,
                                func=mybir.ActivationFunctionType.Identity,
                                bias=pb2_sb[cot],
                            )
                            nc.vector.tensor_add(
                                yo, yo, xt[cot][:, nh * 512 : (nh + 1) * 512]
                            )
                        else:
                            nc.vector.tensor_add(
                                yo, yps, xt[cot][:, nh * 512 : (nh + 1) * 512]
                            )
                        nc.sync.dma_start(
                            out=yap[b, cot * 128 : (cot + 1) * 128, nh * 512 : (nh + 1) * 512],
                            in_=yo,
                        )

                st_exp_denom(0)
                o_accum(0)
                st_exp_denom(1)
                o_accum(1)
                norm_proj(0)
                norm_proj(1)

    _split_multi_waits(nc)
    return nc


_NC_CACHE: dict = {}


def _get_nc(q_bias_nonzero: bool, p_bias_nonzero: bool) -> bass.Bass:
    key = (q_bias_nonzero, p_bias_nonzero)
    if key not in _NC_CACHE:
        _NC_CACHE[key] = build_nc(*key)
    return _NC_CACHE[key]


def kernel(x, gn_scale, gn_bias, qkv_w, qkv_b, proj_w, proj_b, _trace=False):
    from concourse.bass_utils import run_bass_kernel_spmd

    x = np.asarray(x, dtype=np.float32)
    gn_scale = np.asarray(gn_scale, dtype=np.float32)
    gn_bias = np.asarray(gn_bias, dtype=np.float32)
    qkv_w = np.asarray(qkv_w, dtype=np.float32)
    qkv_b = np.asarray(qkv_b, dtype=np.float32)
    proj_w = np.asarray(proj_w, dtype=np.float32)
    proj_b = np.asarray(proj_b, dtype=np.float32)

    qb = qkv_b[:C]
    vb = qkv_b[2 * C : 3 * C]
    # K-bias is softmax-invariant (constant per-query logit shift) -> dropped.
    # V-bias passes linearly through attention (weights sum to 1) -> fold into
    # the proj bias.
    pb2 = proj_w @ vb + proj_b

    q_bias_nonzero = bool(np.any(qb != 0))
    p_bias_nonzero = bool(np.any(pb2 != 0))
    nc = _get_nc(q_bias_nonzero, p_bias_nonzero)

    wqkvT = np.ascontiguousarray(qkv_w.T).astype(BF16_NP)
    pwT = np.ascontiguousarray(proj_w.T).astype(BF16_NP)

    # group-average block matrix over the 128 partitions of a channel tile
    # (two 64-channel groups per tile; 1/64 = 2^-6 is exact in bf16)
    p = np.arange(128)
    bmat = ((p[:, None] // GSIZE) == (p[None, :] // GSIZE)).astype(
        np.float32
    ) / GSIZE

    vecs = np.concatenate(
        [
            gn_scale.reshape(C, 1),
            gn_bias.reshape(C, 1),
            qb.reshape(C, 1),
            pb2.reshape(C, 1).astype(np.float32),
        ],
        axis=1,
    ).reshape(CT, 128, 4)

    xr = x.reshape(B, C, N)
    xrb = xr.astype(BF16_NP)
    shared = {
        "wqkvT": wqkvT,
        "pwT": pwT,
        "vecs": np.ascontiguousarray(vecs),
        "bmat": bmat.astype(BF16_NP),
    }
    in_maps = [
        {
            "x": np.ascontiguousarray(xr[c * NB : (c + 1) * NB]),
            "xb": np.ascontiguousarray(xrb[c * NB : (c + 1) * NB]),
            **shared,
        }
        for c in range(N_CORES)
    ]
    res = run_bass_kernel_spmd(
        nc, in_maps, core_ids=list(range(N_CORES)), trace=_trace
    )
    y = np.concatenate([res.results[c]["y"] for c in range(N_CORES)], axis=0)
    out = y.reshape(B, C, H, W).astype(np.float32)
    if _trace:
        return out, res
    return out



# revision 11
# speedup vs baseline: 1.6974x; 1.6974x over previous
"""AttentionBlock (GroupNorm + single-head self-attention + proj + residual)
Trainium2 Bass/Tile kernel, data-parallel over batch across 8 NeuronCores.

v2: fp8e4 DoubleRow matmuls (2x PE rate) + host-folded weight products that
eliminate half the PSUM->SBUF eviction traffic:

  G = Wq^T Wk  (host, fp32)  ->  S = h^T G h   (Q,K never materialized)
  P = proj_w Wv (host)       ->  y_attn = (P h) Ehat rd  (V,O,proj merged)

Per batch element (C=512, N=1024; all matmuls fp8 DoubleRow, K=256/instr):
  h    = GroupNorm(x)                      [c, n]  fp8, from bf16 x
  Z    = G16 h                             [c, n]  fp8 (16x scale)
  S^T  = Z^T h / (16 sqrt(C))              exp -> Ehat = exp(S^T)/16   fp8
  VP^T = h^T P16^T                         [m, co] fp8
  d    = ones^T Ehat (PE column-sum, replicated over partitions)
  psY  = VP16 Ehat = 16*256^-... = 16*(P h Ehat)    [co, n] PSUM
  y*256 = psY * exp(ln16 - ln d)  + 256*(x + pb2)   -> bf16 out, host /256

Scales: all weight-ish fp8 tensors carry x16 (power of 2, exact) to center
fp8e4's normal range; exp carries 1/16; the net 16/d folds into the
denominator reciprocal, computed as exp(ln16 - ln d) on ACT (one act table
holds ln+exp+identity+copy -> zero table switches).

Engine budget (per core, both batches): PE ~136 DoubleRow MMs ~ 30us;
ACT: GN applies, Z evicts, exps, rdb, half the VP evicts; DVE: bn_stats,
stat chain, 256x residual prep (4x mode), half VP evicts, ymul, yadd.
K-bias is softmax-invariant (dropped); V-bias folds into pb2 = proj_w vb +
proj_b (host); Q-bias, when nonzero, becomes a per-key-partition exp bias
computed from u = Wk^T qb with tiny extra matmuls.
"""

import sys

for _p in ("/opt/trn_rl_repo",):
    if _p not in sys.path:
        sys.path.insert(0, _p)

import math

import ml_dtypes
import numpy as np

import concourse.bass as bass
import concourse.tile as tile
from concourse import mybir
from concourse.vector_clock import ScopedClock, VectorClock

B, C, H, W = 16, 512, 32, 32
N = H * W  # 1024
NUM_GROUPS = 8
EPS = 1e-5
N_CORES = 8
NB = B // N_CORES  # batches per core = 2
CT = C // 128  # channel partition tiles = 4
CP = CT // 2  # DoubleRow channel tile pairs = 2
NT = N // 128  # pixel partition tiles = 8
MP = NT // 2  # pixel tile pairs = 4
NH = N // 512  # free-dim halves = 2
GSIZE = C // NUM_GROUPS  # 64 channels per group
W16 = 16.0
SCALE16 = 1.0 / (math.sqrt(C) * W16)  # exp input scale for S psum
LN16 = math.log(16.0)

F32 = mybir.dt.float32
BF16 = mybir.dt.bfloat16
FP8 = mybir.dt.float8e4
DR = mybir.MatmulPerfMode.DoubleRow
BF16_NP = ml_dtypes.bfloat16
E4_NP = ml_dtypes.float8_e4m3  # TRN FP8_EXP4-compatible (max 240)
AF = mybir.ActivationFunctionType


# --- workaround: this container's walrus accepts only ONE sync wait on the
# SP CTRL drain that TileContext emits at kernel tail; split it into
# single-wait drains.
def _chunked_drain_and_barrier(self, tick_clock, wait_clock):
    gc = tick_clock.global_clock
    ticks = None
    for _k, v in ScopedClock({None: gc}).items():
        ticks = eval(repr(v).replace("VectorClock", ""))
    assert ticks is not None
    n = len(ticks)
    for i in range(n):
        if ticks[i] <= 0:
            continue
        cticks = [ticks[j] if j == i else 0 for j in range(n)]
        drain_inst = self.nc.sync.drain()
        wait_clock.add_sem_waits(
            drain_inst.ins, ScopedClock({None: VectorClock(cticks)})
        )
    self.nc.all_engine_barrier()
    popped = self.nc._tile_sem_poison_stack.pop()
    assert popped is self._sem_poison
    self.nc.clear_and_free_semaphores(list(self.sems.allocated().values()))
    self.nc.all_engine_barrier()


tile.TileContext._drain_and_barrier = _chunked_drain_and_barrier


def _split_multi_waits(nc: bass.Bass, max_waits: int = 1) -> None:
    """Walrus in this container rejects instructions carrying more than one
    sync wait. Hoist excess waits onto same-engine NoOp carriers placed
    immediately before the instruction (same engine queue -> same blocking
    semantics)."""
    n_split = 0
    for f in nc.m.functions:
        for bb in f.blocks:
            insts = bb.instructions
            new = []
            for inst in insts:
                si = inst.sync_info
                if si is not None and len(si.on_wait) > max_waits:
                    waits = list(si.on_wait)
                    keep = waits[-max_waits:]
                    for w in waits[: -max_waits]:
                        nop = mybir.InstNoOp(
                            name=f"{inst.name}-wsplit{n_split}",
                            engine=inst.engine,
                            bass_nofuse=True,
                            sync_info=mybir.SyncInfo(on_wait=[w], on_update=[]),
                        )
                        new.append(nop)
                        n_split += 1
                    inst.sync_info = mybir.SyncInfo(
                        on_wait=keep, on_update=list(si.on_update)
                    )
                new.append(inst)
            insts[:] = new


def build_nc(q_bias_nonzero: bool) -> bass.Bass:
    nc = bass.Bass(trn_type="TRN2")

    xb_d = nc.dram_tensor("xb", [NB, C, N], BF16, kind="ExternalInput")
    g8_d = nc.dram_tensor("g8", [CP, 128, 2, C], FP8, kind="ExternalInput")
    p8_d = nc.dram_tensor("p8", [CP, 128, 2, C], FP8, kind="ExternalInput")
    # per-channel vectors: [:, ct, 0]=gn_scale, 1=gn_bias, 2=256*pb2
    vecs_d = nc.dram_tensor("vecs", [128, CT, 4], F32, kind="ExternalInput")
    # 16*u = 16*Wk^T qb as fp8 pairs for the q-bias correction path
    u8_d = nc.dram_tensor("u8", [128, CT, 1], FP8, kind="ExternalInput")
    # group-average block matrix: bmat[p, p'] = 1/64 if p//64 == p'//64
    bmat_d = nc.dram_tensor("bmat", [128, 128], BF16, kind="ExternalInput")
    ones8_d = nc.dram_tensor("ones8", [128, 2, 128], FP8, kind="ExternalInput")
    y_d = nc.dram_tensor("y", [NB, C, N], BF16, kind="ExternalOutput")

    xbap = xb_d.ap()
    yap = y_d.ap()

    with tile.TileContext(nc) as tc:
        with (
            tc.tile_pool(name="singles", bufs=1) as singles,
            tc.tile_pool(name="xin", bufs=2) as xin,
            tc.tile_pool(name="xpb", bufs=2) as xpb,
            tc.tile_pool(name="stats", bufs=2) as stats,
            tc.tile_pool(name="hp", bufs=2) as hp,
            tc.tile_pool(name="zp", bufs=2) as zp,
            tc.tile_pool(name="vpp", bufs=2) as vpp,
            tc.tile_pool(name="ep", bufs=2) as ep,
            tc.tile_pool(name="yp", bufs=2) as yp,
            tc.tile_pool(name="ps_big", bufs=3, space="PSUM") as ps_big,
            tc.tile_pool(name="ps_dps", bufs=1, space="PSUM") as ps_dps,
        ):
            # ---- tiny constants first on gpsimd (they gate the GN chain)
            vecs = singles.tile([128, CT, 4], F32, tag="vecs")
            nc.gpsimd.dma_start(out=vecs, in_=vecs_d.ap())
            bmat = singles.tile([128, 128], BF16, tag="bmat")
            nc.gpsimd.dma_start(out=bmat, in_=bmat_d.ap())
            ones8 = singles.tile([128, 2, 128], FP8, tag="ones8")
            nc.gpsimd.dma_start(out=ones8, in_=ones8_d.ap())
            u8 = singles.tile([128, CT, 1], FP8, tag="u8")
            if q_bias_nonzero:
                nc.gpsimd.dma_start(out=u8, in_=u8_d.ap())

            # ---- x loads: batch 0 on the two HWDGE rings (gates everything),
            # weights + batch 1 on the gpsimd SWDGE path.
            xt_all = [[None] * CT for _ in range(NB)]
            x_engs = [nc.sync, nc.scalar, nc.sync, nc.scalar]
            for ct in range(CT):
                t = xin.tile([128, N], BF16, tag=f"x{ct}")
                x_engs[ct].dma_start(
                    out=t, in_=xbap[0, ct * 128 : (ct + 1) * 128, :]
                )
                xt_all[0][ct] = t
            g8 = []
            p8 = []
            for cp in range(CP):
                g = singles.tile([128, 2, C], FP8, tag=f"g8_{cp}")
                nc.gpsimd.dma_start(out=g, in_=g8_d.ap()[cp])
                g8.append(g)
            for cp in range(CP):
                p = singles.tile([128, 2, C], FP8, tag=f"p8_{cp}")
                nc.gpsimd.dma_start(out=p, in_=p8_d.ap()[cp])
                p8.append(p)
            for ct in range(CT):
                t = xin.tile([128, N], BF16, tag=f"x{ct}")
                nc.gpsimd.dma_start(
                    out=t, in_=xbap[1, ct * 128 : (ct + 1) * 128, :]
                )
                xt_all[1][ct] = t

            epsb = singles.tile([128, 1], F32, tag="epsb")
            nc.vector.memset(epsb, 1.0 + EPS)
            nl16 = singles.tile([128, 1], F32, tag="nl16")
            nc.vector.memset(nl16, -LN16)
            pl16 = singles.tile([128, 1], F32, tag="pl16")
            nc.vector.memset(pl16, LN16)

            # ---- PE warm-up: HAM unthrottles after ~3.4us of sustained
            # activity; real matmuls can only start once GN stats land.
            warm_rhs = singles.tile([128, 128], BF16, tag="warm_rhs")
            nc.vector.memset(warm_rhs, 0.0)
            warm_lhs = singles.tile([128, 1], BF16, tag="warm_lhs")
            nc.vector.memset(warm_lhs, 0.0)
            warm_ps = ps_dps.tile([1, 128], F32, tag="dps")
            for _wi in range(60):
                nc.tensor.matmul(
                    warm_ps, lhsT=warm_lhs, rhs=warm_rhs, start=True, stop=True
                )

            # =================== GroupNorm (per batch) ====================
            h8_all = [[None] * CP for _ in range(NB)]
            xpb2_all = [[None] * CT for _ in range(NB)]

            def gn_pass(b):
                xt = xt_all[b]
                mv4 = stats.tile([128, CT, 2], F32, tag="mv4")
                for ct in range(CT):
                    st = stats.tile([128, 2, 6], F32, tag=f"st{ct}")
                    for sub in range(2):
                        nc.vector.bn_stats(
                            out=st[:, sub, :],
                            in_=xt[ct][:, sub * 512 : (sub + 1) * 512],
                        )
                    nc.vector.bn_aggr(out=mv4[:, ct, :], in_=st)
                # bf16 stats for the group-average matmul; var carried as
                # (var-1) so bf16 rounding acts on a small value.
                mqb4 = stats.tile([128, 3, CT], BF16, tag="mqb4")
                nc.vector.tensor_copy(out=mqb4[:, 0, :], in_=mv4[:, :, 0])
                nc.vector.tensor_scalar_add(mqb4[:, 1, :], mv4[:, :, 1], -1.0)
                nc.vector.tensor_mul(mqb4[:, 2, :], mv4[:, :, 0], mv4[:, :, 0])
                gps = ps_dps.tile([128, 3, CT], F32, tag="dps")
                nc.tensor.matmul(gps, lhsT=bmat, rhs=mqb4, start=True, stop=True)
                gs = stats.tile([128, 3, CT], F32, tag="gs")
                nc.vector.tensor_copy(out=gs, in_=gps)
                var4 = stats.tile([128, CT], F32, tag="var4")
                m24 = stats.tile([128, CT], F32, tag="m24")
                nc.vector.tensor_add(var4, gs[:, 1, :], gs[:, 2, :])
                nc.vector.tensor_mul(m24, gs[:, 0, :], gs[:, 0, :])
                nc.vector.tensor_sub(var4, var4, m24)
                # rstd = exp(-0.5*ln(var + 1 + eps)) on ACT: stays in the
                # ln/exp act table (no Sqrt table load)
                ln4 = stats.tile([128, CT], F32, tag="ln4")
                rstd4 = stats.tile([128, CT], F32, tag="rstd4")
                nc.scalar.activation(out=ln4, in_=var4, func=AF.Ln, bias=epsb)
                nc.scalar.activation(out=rstd4, in_=ln4, func=AF.Exp, scale=-0.5)
                A4 = stats.tile([128, CT], F32, tag="A4")
                B4 = stats.tile([128, CT], F32, tag="B4")
                nc.vector.tensor_mul(A4, rstd4, vecs[:, :, 0])
                nc.vector.tensor_mul(B4, gs[:, 0, :], A4)
                nc.vector.tensor_sub(B4, vecs[:, :, 1], B4)
                h8 = [
                    hp.tile([128, 2, N], FP8, tag=f"h{cp}", name=f"h8_{cp}")
                    for cp in range(CP)
                ]
                # applies on ACT (scale/bias are per-partition APs)
                for ct in range(CT):
                    nc.scalar.activation(
                        out=h8[ct // 2][:, ct % 2, :],
                        in_=xt[ct],
                        func=AF.Identity,
                        scale=A4[:, ct : ct + 1],
                        bias=B4[:, ct : ct + 1],
                    )
                h8_all[b] = h8

            def xpb2_pass(b):
                # residual prep on DVE (bf16 4x mode): 256*x + 256*pb2
                for ct in range(CT):
                    t = xpb.tile([128, N], BF16, tag=f"xpb{ct}")
                    nc.vector.tensor_scalar(
                        out=t,
                        in0=xt_all[b][ct],
                        scalar1=256.0,
                        scalar2=vecs[:, ct, 2:3],
                        op0=mybir.AluOpType.mult,
                        op1=mybir.AluOpType.add,
                    )
                    xpb2_all[b][ct] = t

            # ================= attention (PE-ordered stages) ================
            z8_all = [None] * NB
            e8_all = [None] * NB
            vp8_all = [None] * NB
            dps_all = [None] * NB
            rdb_all = [None] * NB
            ub_all = [None] * NB

            def z_pass(b):
                h8 = h8_all[b]
                z8 = [
                    zp.tile([128, 2, N], FP8, tag=f"z{cp}", name=f"z8_{cp}")
                    for cp in range(CP)
                ]
                for co in range(CT):
                    ps = ps_big.tile([128, N], F32, tag="big")
                    for cp in range(CP):
                        for nh in range(NH):
                            nc.tensor.matmul(
                                ps[:, nh * 512 : (nh + 1) * 512],
                                lhsT=g8[cp][:, :, co * 128 : (co + 1) * 128],
                                rhs=h8[cp][:, :, nh * 512 : (nh + 1) * 512],
                                start=(cp == 0),
                                stop=(cp == CP - 1),
                                perf_mode=DR,
                            )
                    # split evictions between ACT and DVE to balance load
                    if co % 2 == 0:
                        nc.scalar.copy(out=z8[co // 2][:, co % 2, :], in_=ps)
                    else:
                        nc.vector.tensor_copy(
                            out=z8[co // 2][:, co % 2, :], in_=ps
                        )
                z8_all[b] = z8

            def qbias_pass(b):
                # u = Wk^T qb (host, 16x); c_m = u . h_m added to every logit
                # of key m -> fold into the exp bias below.
                h8 = h8_all[b]
                ub = stats.tile([128, NT, 1], F32, tag="ub")
                for mt in range(NT):
                    ups = ps_dps.tile([128, 1], F32, tag="dps")
                    for cp in range(CP):
                        nc.tensor.matmul(
                            ups,
                            lhsT=h8[cp][:, :, mt * 128 : (mt + 1) * 128],
                            rhs=u8[:, 2 * cp : 2 * cp + 2, :],
                            start=(cp == 0),
                            stop=(cp == CP - 1),
                            perf_mode=DR,
                        )
                    # bias = c_psum * SCALE16 - ln16  (c_psum = 16*u.h)
                    nc.vector.tensor_scalar(
                        out=ub[:, mt, :],
                        in0=ups,
                        scalar1=SCALE16,
                        scalar2=nl16,
                        op0=mybir.AluOpType.mult,
                        op1=mybir.AluOpType.add,
                    )
                ub_all[b] = ub

            def s_pass(b):
                h8 = h8_all[b]
                z8 = z8_all[b]
                e8 = [
                    ep.tile([128, 2, N], FP8, tag=f"e{mp}", name=f"e8_{mp}")
                    for mp in range(MP)
                ]
                for mt in range(NT):
                    ps = ps_big.tile([128, N], F32, tag="big")
                    for cp in range(CP):
                        for nh in range(NH):
                            nc.tensor.matmul(
                                ps[:, nh * 512 : (nh + 1) * 512],
                                lhsT=z8[cp][:, :, mt * 128 : (mt + 1) * 128],
                                rhs=h8[cp][:, :, nh * 512 : (nh + 1) * 512],
                                start=(cp == 0),
                                stop=(cp == CP - 1),
                                perf_mode=DR,
                            )
                    bias = (
                        ub_all[b][:, mt, :] if q_bias_nonzero else nl16
                    )
                    nc.scalar.activation(
                        out=e8[mt // 2][:, mt % 2, :],
                        in_=ps,
                        func=AF.Exp,
                        scale=SCALE16,
                        bias=bias,
                    )
                e8_all[b] = e8

            def vp_pass(b):
                h8 = h8_all[b]
                vp8 = [
                    vpp.tile([128, 2, 512], FP8, tag=f"vp{mp}", name=f"vp8_{mp}")
                    for mp in range(MP)
                ]
                for nt in range(NT):
                    ps = ps_big.tile([128, N], F32, tag="big")
                    for cp in range(CP):
                        nc.tensor.matmul(
                            ps[:, 0:512],
                            lhsT=h8[cp][:, :, nt * 128 : (nt + 1) * 128],
                            rhs=p8[cp],
                            start=(cp == 0),
                            stop=(cp == CP - 1),
                            perf_mode=DR,
                        )
                    # split evictions between ACT and DVE to balance load
                    eng = nc.scalar if nt % 2 == 0 else nc.vector
                    if nt % 2 == 0:
                        eng.copy(out=vp8[nt // 2][:, nt % 2, :], in_=ps[:, 0:512])
                    else:
                        eng.tensor_copy(
                            out=vp8[nt // 2][:, nt % 2, :], in_=ps[:, 0:512]
                        )
                vp8_all[b] = vp8

            def denom_pass(b):
                e8 = e8_all[b]
                dps = ps_dps.tile([128, N], F32, tag="dps")
                for mp in range(MP):
                    for nh in range(NH):
                        nc.tensor.matmul(
                            dps[:, nh * 512 : (nh + 1) * 512],
                            lhsT=ones8,
                            rhs=e8[mp][:, :, nh * 512 : (nh + 1) * 512],
                            start=(mp == 0),
                            stop=(mp == MP - 1),
                            perf_mode=DR,
                        )
                dps_all[b] = dps

            def rdb_pass(b):
                # rdb = 16/d = exp(ln16 - ln d) on ACT
                rdb = stats.tile([128, N], F32, tag="rdb")
                nc.scalar.activation(out=rdb, in_=dps_all[b], func=AF.Ln)
                nc.scalar.activation(
                    out=rdb, in_=rdb, func=AF.Exp, scale=-1.0, bias=pl16
                )
                rdb_all[b] = rdb

            def oproj_pass(b):
                # merged O+proj matmuls; per-co evict chases the accumulation:
                # y*256 = psY * rdb + 256*(x + pb2), written bf16
                vp8 = vp8_all[b]
                e8 = e8_all[b]
                for co in range(CT):
                    ps = ps_big.tile([128, N], F32, tag="big")
                    for mp in range(MP):
                        for nh in range(NH):
                            nc.tensor.matmul(
                                ps[:, nh * 512 : (nh + 1) * 512],
                                lhsT=vp8[mp][:, :, co * 128 : (co + 1) * 128],
                                rhs=e8[mp][:, :, nh * 512 : (nh + 1) * 512],
                                start=(mp == 0),
                                stop=(mp == MP - 1),
                                perf_mode=DR,
                            )
                    ym = yp.tile([128, N], BF16, tag=f"ym{co}")
                    nc.vector.tensor_mul(ym, ps, rdb_all[b])
                    yo = yp.tile([128, N], BF16, tag=f"yo{co}")
                    nc.vector.tensor_add(yo, ym, xpb2_all[b][co])
                    nc.sync.dma_start(
                        out=yap[b, co * 128 : (co + 1) * 128, :], in_=yo
                    )

            # stage schedule: batch 1's independent matmuls fill PE while
            # batch 0's softmax-dependent chain resolves on ACT/DVE
            gn_pass(0)
            if q_bias_nonzero:
                qbias_pass(0)
            z_pass(0)
            gn_pass(1)
            xpb2_pass(0)
            xpb2_pass(1)
            s_pass(0)
            vp_pass(0)
            denom_pass(0)
            rdb_pass(0)
            if q_bias_nonzero:
                qbias_pass(1)
            z_pass(1)
            oproj_pass(0)
            s_pass(1)
            vp_pass(1)
            denom_pass(1)
            rdb_pass(1)
            oproj_pass(1)

    _split_multi_waits(nc)
    return nc


_NC_CACHE: dict = {}


def _get_nc(q_bias_nonzero: bool) -> bass.Bass:
    key = (q_bias_nonzero,)
    if key not in _NC_CACHE:
        _NC_CACHE[key] = build_nc(*key)
    return _NC_CACHE[key]


def _pack_pairs(wT16: np.ndarray) -> np.ndarray:
    """[C, F] (c-major rows) -> [CP, 128, 2, F] DoubleRow pair layout,
    c = 256*cp + 128*j + p."""
    F = wT16.shape[1]
    return np.ascontiguousarray(
        wT16.reshape(CP, 2, 128, F).transpose(0, 2, 1, 3)
    )


def _q8(a: np.ndarray, name: str) -> np.ndarray:
    m = float(np.abs(a).max()) if a.size else 0.0
    assert m < 239.0, f"fp8 overflow in {name}: {m}"
    return a.astype(E4_NP)


def kernel(x, gn_scale, gn_bias, qkv_w, qkv_b, proj_w, proj_b, _trace=False):
    from concourse.bass_utils import run_bass_kernel_spmd

    x = np.asarray(x, dtype=np.float32)
    gn_scale = np.asarray(gn_scale, dtype=np.float32)
    gn_bias = np.asarray(gn_bias, dtype=np.float32)
    qkv_w = np.asarray(qkv_w, dtype=np.float32)
    qkv_b = np.asarray(qkv_b, dtype=np.float32)
    proj_w = np.asarray(proj_w, dtype=np.float32)
    proj_b = np.asarray(proj_b, dtype=np.float32)

    Wq, Wk, Wv = qkv_w[:C], qkv_w[C : 2 * C], qkv_w[2 * C :]
    qb = qkv_b[:C]
    vb = qkv_b[2 * C : 3 * C]
    # K-bias shifts all logits of a query equally -> softmax-invariant.
    # V-bias passes linearly through attention -> fold into proj bias.
    pb2 = proj_w @ vb + proj_b
    # Q-bias: S += (Wk^T qb) . h_m, a per-key shift handled on-chip via u.
    u = Wk.T @ qb

    G = Wq.T @ Wk  # [C, C]
    P = proj_w @ Wv  # [C, C]
    # pair layouts want contraction dim c as rows: g8[cp,p,j,i] = 16*G[i, c],
    # c = 256*cp + 128*j + p -> pack G^T / P^T
    g8 = _pack_pairs(_q8(G.T * W16, "G").view(np.uint8)).view(E4_NP)
    p8 = _pack_pairs(_q8(P.T * W16, "P").view(np.uint8)).view(E4_NP)

    q_bias_nonzero = bool(np.any(qb != 0))
    nc = _get_nc(q_bias_nonzero)

    vecs = np.zeros((128, CT, 4), dtype=np.float32)
    vecs[:, :, 0] = gn_scale.reshape(CT, 128).T
    vecs[:, :, 1] = gn_bias.reshape(CT, 128).T
    vecs[:, :, 2] = 256.0 * pb2.reshape(CT, 128).T
    u8 = np.ascontiguousarray(
        _q8((W16 * u).reshape(CT, 128).T, "u")[:, :, None]
        if q_bias_nonzero
        else np.zeros((128, CT, 1), dtype=np.float32).astype(E4_NP)
    )

    p_ = np.arange(128)
    bmat = ((p_[:, None] // GSIZE) == (p_[None, :] // GSIZE)).astype(
        np.float32
    ) / GSIZE

    ones8 = np.ones((128, 2, 128), dtype=np.float32).astype(E4_NP)

    xrb = x.reshape(B, C, N).astype(BF16_NP)
    shared = {
        "g8": g8,
        "p8": p8,
        "vecs": vecs,
        "bmat": bmat.astype(BF16_NP),
        "ones8": ones8,
        "u8": u8,
    }
    in_maps = [
        {"xb": np.ascontiguousarray(xrb[c * NB : (c + 1) * NB]), **shared}
        for c in range(N_CORES)
    ]
    res = run_bass_kernel_spmd(
        nc, in_maps, core_ids=list(range(N_CORES)), trace=_trace
    )
    y = np.concatenate([res.results[c]["y"] for c in range(N_CORES)], axis=0)
    out = (y.astype(np.float32) / 256.0).reshape(B, C, H, W)
    if _trace:
        return out, res
    return out
